# revision 3
# baseline (speedup 1.0000x reference)
"""Trainium2 Bass kernel for CompressDCT (blockwise 8x8 2D DCT + quantize).

Reference computation (encoder, the graded path):
    X = einsum('ij,ncpjqk,lk->ncpiql', D, x_blocks, D)   # D @ block @ D.T
    X = clip(round(X / q_table), -128, 127)              # q_table == ones
Decoder path (is_encoder == 0):
    out = D.T @ (block * q_table) @ D

Strategy: pure data parallel over 8 NeuronCores; each core processes 128
of the 1024 (N*C) 256x256 images.

Per-core kernel, per [128, 256] chunk (half of one image), with
K = kron(I_16, D) block-diagonal [128,128] and C = K.T (encoder):
    MM1: Yt = matmul(lhsT=chunk_tile, rhs=C)  -> chunk.T @ C  (= (K@chunk).T)
    copy Yt PSUM -> SBUF (DVE)
    MM2: Z  = matmul(lhsT=Yt,         rhs=C)  -> (K@chunk) @ K.T
    quantize: ACT copy fp32 -> int8 (hardware does round-half-even +
              saturation to [-128,127], exactly matching round + clip)
    DMA out int8 (4x smaller output traffic; exact since values are small ints)

The matmuls run in fp32 (PE fp32 mode). No cross-core communication.
"""
import os
import sys

import numpy as np

try:
    import concourse.bass as bass  # noqa: F401
except ImportError:
    sys.path.insert(0, "/opt/trn_rl_repo")

import concourse.bacc as bacc
import concourse.tile as tile
from concourse import mybir
from concourse.bass_utils import run_bass_kernel_spmd

BLOCK = 8
N_CORES = 8
# Full input: (16, 64, 256, 256) fp32. Shard along N: 2 N x 64 C = 128 images/core.
IMGS_PER_CORE = 128
H = W = 256

_CACHE = {}
LAST_RESULTS = None  # BassKernelResults of the most recent run (for profiling)
TRACE = False


def _dct_mat():
    # Identical arithmetic to the reference's _dct_mat (fp64 -> fp32 cast).
    i = np.arange(BLOCK)
    k = np.arange(BLOCK)[:, None]
    D = np.cos(np.pi * (2 * i + 1) * k / (2 * BLOCK))
    s = np.full((BLOCK, 1), np.sqrt(2.0 / BLOCK))
    s[0, 0] = np.sqrt(1.0 / BLOCK)
    return (D * s).astype(np.float32)


def _build(encoder: bool, include_q: bool, repeat: int = 0):
    """repeat=0: straight-line kernel (graded path). repeat>0: wrap the body
    in a For_i(0, repeat) hardware loop — used only for differential timing."""
    nc = bacc.Bacc("TRN2", target_bir_lowering=False, debug=False)
    dt = mybir.dt

    x_in = nc.dram_tensor(
        "x", [IMGS_PER_CORE, H, W], dt.float32, kind="ExternalInput"
    ).ap()
    c_in = nc.dram_tensor("kt", [128, 128], dt.float32, kind="ExternalInput").ap()
    if include_q:
        q_in = nc.dram_tensor("rq", [128, 256], dt.float32, kind="ExternalInput").ap()
    odt = dt.int8 if encoder else dt.float32
    out = nc.dram_tensor("out", [IMGS_PER_CORE, H, W], odt, kind="ExternalOutput").ap()

    n_chunks = IMGS_PER_CORE * 2  # two [128, 256] row-halves per image

    from contextlib import ExitStack

    with tile.TileContext(nc) as tc:
        with (
            tc.tile_pool(name="const", bufs=1) as cpool,
            tc.tile_pool(name="pin", bufs=4) as pin,
            tc.tile_pool(name="py", bufs=4) as py,
            tc.tile_pool(name="pout", bufs=4) as pout,
            tc.tile_pool(name="psy", bufs=4, space="PSUM") as psy,
            tc.tile_pool(name="psz", bufs=4, space="PSUM") as psz,
            ExitStack() as lp,
        ):
            t_c = cpool.tile([128, 128], dt.float32)
            nc.sync.dma_start(t_c[:], c_in[:])
            if include_q:
                t_q = cpool.tile([128, 256], dt.float32)
                nc.sync.dma_start(t_q[:], q_in[:])

            if repeat:
                lp.enter_context(tc.For_i(0, repeat, 1))

            for chunk in range(n_chunks):
                img, half = divmod(chunk, 2)
                h0 = half * 128

                t_in = pin.tile([128, 256], dt.float32)
                nc.sync.dma_start(t_in[:], x_in[img, h0 : h0 + 128, :])

                src = t_in
                if not encoder and include_q:
                    # decoder pre-multiplies blocks by q
                    t_xq = pin.tile([128, 256], dt.float32, tag="xq")
                    nc.vector.tensor_mul(t_xq[:], t_in[:], t_q[:])
                    src = t_xq

                p_y = psy.tile([128, 256], dt.float32)
                nc.tensor.matmul(
                    p_y[:, 0:128], src[:, 0:128], t_c[:], start=True, stop=True
                )
                nc.tensor.matmul(
                    p_y[:, 128:256], src[:, 128:256], t_c[:], start=True, stop=True
                )
                t_y = py.tile([128, 256], dt.float32)
                nc.vector.tensor_copy(t_y[:], p_y[:])

                p_z = psz.tile([128, 256], dt.float32)
                nc.tensor.matmul(
                    p_z[:, 0:128], t_y[:, 0:128], t_c[:], start=True, stop=True
                )
                nc.tensor.matmul(
                    p_z[:, 128:256], t_y[:, 128:256], t_c[:], start=True, stop=True
                )

                if encoder:
                    t_o = pout.tile([128, 256], dt.int8)
                    if include_q:
                        # X / q, then round+clip via the int8 cast
                        t_m = py.tile([128, 256], dt.float32, tag="m")
                        nc.vector.tensor_mul(t_m[:], p_z[:], t_q[:])
                        nc.scalar.copy(t_o[:], t_m[:])
                    else:
                        nc.scalar.copy(t_o[:], p_z[:])
                else:
                    t_o = pout.tile([128, 256], dt.float32)
                    nc.scalar.copy(t_o[:], p_z[:])

                nc.sync.dma_start(out[img, h0 : h0 + 128, :], t_o[:])

    nc.compile()
    return nc


def _get(encoder: bool, include_q: bool):
    key = (encoder, include_q)
    if key not in _CACHE:
        _CACHE[key] = _build(encoder, include_q)
    return _CACHE[key]


def kernel(x, q_table, is_encoder):
    global LAST_RESULTS
    x = np.ascontiguousarray(np.asarray(x, dtype=np.float32))
    q = np.asarray(q_table, dtype=np.float32)
    enc = bool(int(np.asarray(is_encoder)))
    include_q = not np.all(q == 1.0)

    N, C, H_, W_ = x.shape
    assert (H_, W_) == (H, W) and N * C == N_CORES * IMGS_PER_CORE

    D = _dct_mat()
    K = np.kron(np.eye(16, dtype=np.float32), D)  # [128, 128] block-diagonal
    const = np.ascontiguousarray((K.T if enc else K), dtype=np.float32)

    shards = x.reshape(N_CORES, IMGS_PER_CORE, H, W)
    in_maps = []
    for c in range(N_CORES):
        m = {"x": shards[c], "kt": const}
        if include_q:
            qt = np.tile(q, (16, 32)).astype(np.float32)  # [128, 256]
            m["rq"] = np.ascontiguousarray(1.0 / qt if enc else qt)
        in_maps.append(m)

    nc = _get(enc, include_q)
    res = run_bass_kernel_spmd(
        nc, in_maps, list(range(N_CORES)), trace=TRACE or bool(os.environ.get("KERNEL_TRACE"))
    )
    LAST_RESULTS = res

    o = np.stack([res.results[c]["out"] for c in range(N_CORES)])
    return o.reshape(N, C, H, W).astype(np.float32)


# revision 11
# speedup vs baseline: 1.6453x; 1.6453x over previous
"""Trainium2 Bass kernel for CompressDCT (blockwise 8x8 2D DCT + quantize).

Reference computation (encoder, the graded path):
    X = einsum('ij,ncpjqk,lk->ncpiql', D, x_blocks, D)   # D @ block @ D.T
    X = clip(round(X / q_table), -128, 127)              # q_table == ones
Decoder path (is_encoder == 0):
    out = D.T @ (block * q_table) @ D

Strategy: pure data parallel over 8 NeuronCores; each core processes 128
of the 1024 (N*C) 256x256 images.

Per-core kernel, per [128, 256] chunk (half of one image), with
K = kron(I_16, D) block-diagonal [128,128] and C = K.T (encoder):
    MM1: Yt = matmul(lhsT=chunk_tile, rhs=C)  -> chunk.T @ C  (= (K@chunk).T)
    copy Yt PSUM -> SBUF (DVE)
    MM2: Z  = matmul(lhsT=Yt,         rhs=C)  -> (K@chunk) @ K.T
    quantize: ACT copy fp32 -> int8 (hardware does round-half-even +
              saturation to [-128,127], exactly matching round + clip)
    DMA out int8 (4x smaller output traffic; exact since values are small ints)

The matmuls run in fp32 (PE fp32 mode). No cross-core communication.
"""
import os
import sys

import numpy as np

try:
    import concourse.bass as bass  # noqa: F401
except ImportError:
    sys.path.insert(0, "/opt/trn_rl_repo")

import concourse.bacc as bacc
import concourse.tile as tile
from concourse import mybir
from concourse.bass_utils import run_bass_kernel_spmd

BLOCK = 8
N_CORES = 8
# Full input: (16, 64, 256, 256) fp32. Shard along N: 2 N x 64 C = 128 images/core.
IMGS_PER_CORE = 128
H = W = 256

_CACHE = {}
LAST_RESULTS = None  # BassKernelResults of the most recent run (for profiling)
TRACE = False


def _dct_mat():
    # Identical arithmetic to the reference's _dct_mat (fp64 -> fp32 cast).
    i = np.arange(BLOCK)
    k = np.arange(BLOCK)[:, None]
    D = np.cos(np.pi * (2 * i + 1) * k / (2 * BLOCK))
    s = np.full((BLOCK, 1), np.sqrt(2.0 / BLOCK))
    s[0, 0] = np.sqrt(1.0 / BLOCK)
    return (D * s).astype(np.float32)


def _build(encoder: bool, include_q: bool, repeat: int = 0, knobs: dict | None = None):
    """repeat=0: straight-line kernel (graded path). repeat>0: wrap the body
    in a For_i(0, repeat) hardware loop — used only for differential timing."""
    kn = {
        "bufs_in": 4, "bufs_y": 4, "bufs_out": 4, "bufs_psy": 4, "bufs_psz": 4,
        "split_copy": False,  # split Yt copy DVE/ACT by halves
        "split_cast": False,  # split final cast ACT/DVE by halves
        "stagger": 0,  # software-pipeline depth: stage-2 of chunk i emitted
                       # after stage-1 of chunk i+stagger
    }
    kn.update(knobs or {})
    nc = bacc.Bacc("TRN2", target_bir_lowering=False, debug=False)
    dt = mybir.dt

    x_in = nc.dram_tensor(
        "x", [IMGS_PER_CORE, H, W], dt.float32, kind="ExternalInput"
    ).ap()
    c_in = nc.dram_tensor("kt", [128, 128], dt.float32, kind="ExternalInput").ap()
    if include_q:
        q_in = nc.dram_tensor("rq", [128, 256], dt.float32, kind="ExternalInput").ap()
    odt = dt.int8 if encoder else dt.float32
    out = nc.dram_tensor(
        "out", [IMGS_PER_CORE // 2, 128, 1024], odt, kind="ExternalOutput"
    ).ap()

    n_chunks = IMGS_PER_CORE * 2  # two [128, 256] row-halves per image

    from contextlib import ExitStack

    with tile.TileContext(nc) as tc:
        with (
            tc.tile_pool(name="const", bufs=1) as cpool,
            tc.tile_pool(name="pin", bufs=kn["bufs_in"]) as pin,
            tc.tile_pool(name="py", bufs=kn["bufs_y"]) as py,
            tc.tile_pool(name="pout", bufs=kn["bufs_out"]) as pout,
            tc.tile_pool(name="psy", bufs=kn["bufs_psy"], space="PSUM") as psy,
            tc.tile_pool(name="psz", bufs=kn["bufs_psz"], space="PSUM") as psz,
            ExitStack() as lp,
        ):
            t_c = cpool.tile([128, 128], dt.float32)
            nc.sync.dma_start(t_c[:], c_in[:])
            if include_q:
                t_q = cpool.tile([128, 256], dt.float32)
                nc.sync.dma_start(t_q[:], q_in[:])

            if repeat:
                lp.enter_context(tc.For_i(0, repeat, 1))

            # One input DMA per image: [128, 512] tile, cols (h, w) with
            # h in {0,1} row-halves. One output DMA per image pair:
            # [128, 1024] int8 tile of 4 quarter-chunks.
            in_tiles = {}
            out_tiles = {}

            def stage1(chunk):
                img, half = divmod(chunk, 2)
                if half == 0:
                    t_in = pin.tile([128, 512], dt.float32, tag="t_in")
                    nc.sync.dma_start(
                        t_in[:].rearrange("p (h w) -> p h w", h=2),
                        x_in[img].rearrange("(h p) w -> p h w", h=2),
                    )
                    in_tiles[img] = t_in
                t_in = in_tiles[img]
                c0 = half * 256

                src = t_in
                s0 = c0
                if not encoder and include_q:
                    # decoder pre-multiplies blocks by q
                    t_xq = pin.tile([128, 256], dt.float32, tag="xq")
                    nc.vector.tensor_mul(t_xq[:], t_in[:, c0 : c0 + 256], t_q[:])
                    src, s0 = t_xq, 0

                p_y = psy.tile([128, 256], dt.float32, tag="p_y")
                nc.tensor.matmul(
                    p_y[:, 0:128], src[:, s0 : s0 + 128], t_c[:], start=True, stop=True
                )
                nc.tensor.matmul(
                    p_y[:, 128:256], src[:, s0 + 128 : s0 + 256], t_c[:],
                    start=True, stop=True
                )
                t_y = py.tile([128, 256], dt.float32, tag="t_y")
                if kn["split_copy"]:
                    nc.vector.tensor_copy(t_y[:, 0:128], p_y[:, 0:128])
                    nc.scalar.copy(t_y[:, 128:256], p_y[:, 128:256])
                else:
                    nc.vector.tensor_copy(t_y[:], p_y[:])
                return t_y

            def stage2(chunk, t_y):
                img, half = divmod(chunk, 2)
                pair, quarter = divmod(chunk, 4)

                p_z = psz.tile([128, 256], dt.float32, tag="p_z")
                nc.tensor.matmul(
                    p_z[:, 0:128], t_y[:, 0:128], t_c[:], start=True, stop=True
                )
                nc.tensor.matmul(
                    p_z[:, 128:256], t_y[:, 128:256], t_c[:], start=True, stop=True
                )

                if quarter == 0:
                    out_tiles[pair] = pout.tile(
                        [128, 1024], dt.int8 if encoder else dt.float32,
                        tag="t_o", name=f"t_o_{pair}"
                    )
                t_o = out_tiles[pair]
                q0 = quarter * 256

                if encoder:
                    if include_q:
                        # X / q, then round+clip via the int8 cast
                        t_m = py.tile([128, 256], dt.float32, tag="m")
                        nc.vector.tensor_mul(t_m[:], p_z[:], t_q[:])
                        nc.scalar.copy(t_o[:, q0 : q0 + 256], t_m[:])
                    elif kn["split_cast"]:
                        nc.scalar.copy(t_o[:, q0 : q0 + 128], p_z[:, 0:128])
                        nc.vector.tensor_copy(
                            t_o[:, q0 + 128 : q0 + 256], p_z[:, 128:256]
                        )
                    else:
                        nc.scalar.copy(t_o[:, q0 : q0 + 256], p_z[:])
                else:
                    nc.scalar.copy(t_o[:, q0 : q0 + 256], p_z[:])

                if quarter == 3:
                    nc.sync.dma_start(out[pair], t_o[:])
                    del out_tiles[pair]

            S = kn["stagger"]
            pending = []
            for chunk in range(n_chunks):
                pending.append((chunk, stage1(chunk)))
                if len(pending) > S:
                    c2, ty2 = pending.pop(0)
                    stage2(c2, ty2)
            for c2, ty2 in pending:
                stage2(c2, ty2)

    nc.compile()
    return nc


def _get(encoder: bool, include_q: bool):
    key = (encoder, include_q)
    if key not in _CACHE:
        _CACHE[key] = _build(encoder, include_q)
    return _CACHE[key]


def kernel(x, q_table, is_encoder):
    global LAST_RESULTS
    x = np.ascontiguousarray(np.asarray(x, dtype=np.float32))
    q = np.asarray(q_table, dtype=np.float32)
    enc = bool(int(np.asarray(is_encoder)))
    include_q = not np.all(q == 1.0)

    N, C, H_, W_ = x.shape
    assert (H_, W_) == (H, W) and N * C == N_CORES * IMGS_PER_CORE

    D = _dct_mat()
    K = np.kron(np.eye(16, dtype=np.float32), D)  # [128, 128] block-diagonal
    const = np.ascontiguousarray((K.T if enc else K), dtype=np.float32)

    shards = x.reshape(N_CORES, IMGS_PER_CORE, H, W)
    in_maps = []
    for c in range(N_CORES):
        m = {"x": shards[c], "kt": const}
        if include_q:
            qt = np.tile(q, (16, 32)).astype(np.float32)  # [128, 256]
            m["rq"] = np.ascontiguousarray(1.0 / qt if enc else qt)
        in_maps.append(m)

    nc = _get(enc, include_q)
    res = run_bass_kernel_spmd(
        nc, in_maps, list(range(N_CORES)), trace=TRACE or bool(os.environ.get("KERNEL_TRACE"))
    )
    LAST_RESULTS = res

    o = np.stack([res.results[c]["out"] for c in range(N_CORES)])
    # [core, pair, p, (img_in_pair, half, w)] -> [core, pair, img_in_pair, half, p, w]
    o = o.reshape(N_CORES, IMGS_PER_CORE // 2, 128, 2, 2, 256)
    o = o.transpose(0, 1, 3, 4, 2, 5)
    return np.ascontiguousarray(o).reshape(N, C, H, W).astype(np.float32)
